# revision 1
# baseline (speedup 1.0000x reference)
"""Trainium2 Bass kernel for nn_Adapter2 (dense_cnn), v2: fp8 mixed precision.

Strategy (8 cores, data-parallel over clips, zero collectives), per core:
  xT [768, 197*32] b-major token order (b, l, t) per h-row tile, bf16 + fp8.

  Branch precision split (output norm is dominated by the mlp/gelu branch):
    - main (fc1 -> conv_t -> fc2) and offset (diff -> fc1 -> spconv3x3 -> fc2)
      run in fp8e4m3: matmul A via DoubleRow (K=256/instr), z stored fp8 x32,
      w2 fp8 x32; their PSUM contribution lands x1024.
    - mlp branch (gelu) runs bf16 end-to-end with its w2 pre-scaled x1024 so
      ALL branches accumulate in one PSUM; eviction rescales by 1/1024.
  Bias edge corrections (conv_t t-edges, CLS masking, offset biases) ride
  three constant aux z-rows with host-computed w2 rows: zero per-tile ops
  (the baseline spent 130us of GpSimd on these).

  Middle (per tile): conv_t = center ACT + 4 shifted DVE taps accumulating in
  fp8; the 64-wide chunks (fc1 ca128:192, off ca0:64) are b-packed onto 128
  partitions so their ops are 3D/half-width; spatial conv = 9 shifted-AP taps
  per chunk accumulated in bf16 with the last tap writing fp8.
"""
import sys

if "/opt/trn_rl_repo" not in sys.path:
    sys.path.insert(0, "/opt/trn_rl_repo")

import numpy as np
import ml_dtypes

import concourse.bass as bass
import concourse.mybir as mybir
from concourse.tile import TileContext
from concourse import bass_utils, bacc

F32 = mybir.dt.float32
BF16 = mybir.dt.bfloat16
F8 = mybir.dt.float8e4
AF = mybir.ActivationFunctionType
OP = mybir.AluOpType
PM = mybir.MatmulPerfMode.DoubleRow

C = 768
CA = 192
L = 197
T = 16
NCORES = 8
NL = 32                      # N-columns per core (2 clips x 16 frames)
NCOLS = L * NL               # 6304
HGRID = 14
PADW = 16
SW = 32.0                    # fp8 weight scale
SZ = 32.0                    # fp8 z scale (= SW, forced by diff path)
G = SW * SZ                  # net fp8-psum scale (gelu w2 pre-scaled by G)

# token tiles: tile 0 = l 0..14 (CLS + h-row 0), tiles 1..13 = h-rows 1..13
TILES = [(0, 15)] + [(1 + 14 * k, 14) for k in range(1, 14)]
TILE_C0 = [0]
for _, _nl in TILES:
    TILE_C0.append(TILE_C0[-1] + _nl * NL)

NSCAL = 38
TAPS = [(dh, dw) for dh in (-1, 0, 1) for dw in (-1, 0, 1)]

PPADU_COLS = 2 * 16 * PADW * T   # (b, h, w, t) for off ca 64:192 (128p)
PPADP_COLS = 16 * PADW * T       # packed (h, w, t) for off ca 0:64


def _dup(v):
    return np.concatenate([v, v])


def _pack_scalars(conv_w, conv_b, fc1_b, mlp_in_b, off_fc1_b, off_conv_w,
                  fc2_b, mlp_out_b):
    s = np.zeros((128, NSCAL), np.float32)
    w0, w1, w2 = conv_w[:, 0, 0], conv_w[:, 0, 1], conv_w[:, 0, 2]
    wsum_b = (w0 + w1 + w2) * fc1_b + conv_b
    wsp = off_conv_w[:, 0, 0, :, :]          # (CA, 3, 3)
    # conv_t chunk A (fc1 ca 0:128)
    s[:, 0] = w0[:128]; s[:, 1] = w1[:128]; s[:, 2] = w2[:128]
    s[:, 3] = SZ * wsum_b[:128]
    # conv_t chunk B packed (fc1 ca 128:192 on both halves)
    s[:, 4] = _dup(w0[128:]); s[:, 5] = _dup(w1[128:]); s[:, 6] = _dup(w2[128:])
    s[:, 7] = _dup(SZ * wsum_b[128:])
    # gelu biases (Silu(1.702 y + 1.702 b) = 1.702 qgelu(y+b))
    s[:, 8] = 1.702 * mlp_in_b[0:128]
    s[:64, 9] = 1.702 * mlp_in_b[128:192]
    # diff biases (z-scaled); also the ppad t=0 planes
    s[:, 10] = SZ * off_fc1_b[64:192]
    s[:, 11] = _dup(SZ * off_fc1_b[0:64])
    # spconv taps: U = off ca 64:192, P = packed off ca 0:64
    for i, (dh, dw) in enumerate(TAPS):
        s[:, 12 + i] = wsp[64:192, dh + 1, dw + 1]
        s[:, 21 + i] = _dup(wsp[0:64, dh + 1, dw + 1])
    # B eviction bias per out chunk
    bb = fc2_b + mlp_out_b
    for m in range(6):
        s[:, 32 + m] = bb[m * 128:(m + 1) * 128]
    return s


def _aux_patterns():
    """Constant aux z-rows [3, 480+448] bf16: (l>=1), (t==0), (t==15)."""
    out = np.zeros((3, 480 + 448), np.float32)
    for w0, nl in ((0, 15), (480, 14)):
        loff = 1 if nl == 15 else 0
        for b in range(2):
            for l in range(nl):
                for t in range(T):
                    c = w0 + b * nl * T + l * T + t
                    out[0, c] = 0.0 if (nl == 15 and l == 0) else 1.0
                    out[1, c] = 1.0 if t == 0 else 0.0
                    out[2, c] = 1.0 if t == T - 1 else 0.0
    return out.astype(ml_dtypes.bfloat16)


def build_kernel(diff_bias_zero=True):
    nc = bacc.Bacc("TRN2", target_bir_lowering=False, debug=False,
                   num_devices=NCORES)
    xt_d = nc.declare_dram_parameter("xt", [C, NCOLS], BF16, isOutput=False)
    x8_d = nc.declare_dram_parameter("x8", [128, 6, NCOLS], F8, isOutput=False)
    w1b_d = nc.declare_dram_parameter("w1b", [C, 192], BF16, isOutput=False)
    w18_d = nc.declare_dram_parameter("w18", [128, 6, 384], F8, isOutput=False)
    w2a_d = nc.declare_dram_parameter("w2a", [128, 2, C], F8, isOutput=False)
    w2b8_d = nc.declare_dram_parameter("w2b8", [128, C], F8, isOutput=False)
    w2g0_d = nc.declare_dram_parameter("w2g0", [128, C], BF16, isOutput=False)
    w2g1_d = nc.declare_dram_parameter("w2g1", [67, C], BF16, isOutput=False)
    aux_d = nc.declare_dram_parameter("auxz", [3, 480 + 448], BF16,
                                      isOutput=False)
    sc_d = nc.declare_dram_parameter("scal", [128, NSCAL], F32, isOutput=False)
    out_d = nc.declare_dram_parameter("out", [C, NCOLS], BF16, isOutput=True)

    with TileContext(nc) as tc:
        with (
            tc.tile_pool(name="const", bufs=1) as cpool,
            tc.tile_pool(name="xin", bufs=1) as xpool,
            tc.tile_pool(name="z", bufs=4) as zpool,
            tc.tile_pool(name="tmp", bufs=2) as tpool,
            tc.tile_pool(name="osb", bufs=6) as opool_sb,
            tc.tile_pool(name="ypsum", bufs=1, space="PSUM") as ypool,
            tc.tile_pool(name="ypsum2", bufs=1, space="PSUM") as ypool2,
            tc.tile_pool(name="opsum", bufs=3, space="PSUM") as opool,
        ):
            scal = cpool.tile([128, NSCAL], F32, name="scal")
            nc.sync.dma_start(out=scal[:], in_=sc_d[:])

            xt_sb = [[None] * 6 for _ in TILES]     # bf16 ktiles per tile
            x8_sb = [None] * len(TILES)             # fp8 [128, 6, w] per tile

            def load_group(ks):
                c0 = TILE_C0[ks[0]]
                c1 = TILE_C0[ks[-1] + 1]
                gw = c1 - c0
                t8 = xpool.tile([128, 6, gw], F8, name=f"x8g_{ks[0]}")
                nc.sync.dma_start(out=t8[:], in_=x8_d[:, :, c0:c1])
                for k in ks:
                    o = TILE_C0[k] - c0
                    w = TILES[k][1] * NL
                    x8_sb[k] = t8[:, :, o:o + w]
                for i in range(6):
                    t = xpool.tile([128, gw], BF16, name=f"xtg_{ks[0]}_{i}")
                    nc.sync.dma_start(out=t[:],
                                      in_=xt_d[i * 128:(i + 1) * 128, c0:c1])
                    for k in ks:
                        o = TILE_C0[k] - c0
                        w = TILES[k][1] * NL
                        xt_sb[k][i] = t[:, o:o + w]

            load_group([0, 1])
            # weights
            w18 = cpool.tile([128, 6, 384], F8, name="w18")
            nc.scalar.dma_start(out=w18[:], in_=w18_d[:])
            w1b = []
            for i in range(6):
                t = cpool.tile([128, 192], BF16, name=f"w1b_{i}")
                dma = nc.scalar if i % 2 == 0 else nc.sync
                dma.dma_start(out=t[:], in_=w1b_d[i * 128:(i + 1) * 128, :])
                w1b.append(t)
            w2a = cpool.tile([128, 2, C], F8, name="w2a")
            nc.gpsimd.dma_start(out=w2a[:], in_=w2a_d[:])
            w2b8 = cpool.tile([128, C], F8, name="w2b8")
            nc.gpsimd.dma_start(out=w2b8[:], in_=w2b8_d[:])
            w2g0 = cpool.tile([128, C], BF16, name="w2g0")
            nc.gpsimd.dma_start(out=w2g0[:], in_=w2g0_d[:])
            w2g1 = cpool.tile([67, C], BF16, name="w2g1")
            nc.gpsimd.dma_start(out=w2g1[:], in_=w2g1_d[:])

            # padded diff grids (persistent; guards stay zero)
            ppadU = cpool.tile([128, PPADU_COLS], BF16, name="ppadU")
            ppadP = cpool.tile([128, PPADP_COLS], BF16, name="ppadP")
            nc.gpsimd.memset(ppadU[:], 0.0)
            nc.gpsimd.memset(ppadP[:], 0.0)
            pu5 = ppadU[:, :].rearrange("p (b h w t) -> p b h w t",
                                        b=2, h=16, w=PADW)
            pp4 = ppadP[:, :].rearrange("p (h w t) -> p h w t", h=16, w=PADW)
            for b in (0, 1):
                t0u = pu5[:, b, 1:15, 1:15, 0:1]
                nc.scalar.activation(t0u, t0u, AF.Identity,
                                     bias=scal[:, 10:11], scale=0.0)
            t0p = pp4[:, 1:15, 1:15, 0:1]
            nc.scalar.activation(t0p, t0p, AF.Identity,
                                 bias=scal[:, 11:12], scale=0.0)

            # gelu-B z tiles with constant aux rows 64:67 (manual 4-buffer
            # rotation — z tiles live 3 iterations — so the aux rows are
            # written once at init)
            zg1_t0 = cpool.tile([67, 480], BF16, name="zg1_t0")
            nc.sync.dma_start(out=zg1_t0[64:67, :], in_=aux_d[:, 0:480])
            zg1_rot = []
            for r in range(4):
                t = cpool.tile([67, 448], BF16, name=f"zg1_{r}")
                nc.sync.dma_start(out=t[64:67, :], in_=aux_d[:, 480:928])
                zg1_rot.append(t)

            load_group([2, 3, 4, 5])
            load_group([6, 7, 8, 9])
            load_group([10, 11, 12, 13])

            def col(j, r0=0, r1=128):
                return scal[r0:r1, j:j + 1]

            # padded-grid views for diff/taps
            puv = ppadU[:, :].rearrange("p (b l t) -> p b l t", b=2, t=T)
            ppv = ppadP[:, :].rearrange("p (l t) -> p l t", t=T)

            z_tiles = [None] * 14   # (zf01, zf2, zg0, zg1)

            def emit_A(k):
                nl = TILES[k][1]
                w = nl * NL
                ys = []
                # fp8 DoubleRow: out chunks [fc1 0:128], [fc1 128:192|off 0:64],
                # [off 64:192]
                for m in range(3):
                    pool_m = ypool2 if m == 0 else ypool
                    yt = pool_m.tile([128, w], F32, name=f"y{m}")
                    for j in range(3):
                        nc.tensor.matmul(
                            yt[:, :], w18[:, 2 * j:2 * j + 2,
                                          m * 128:(m + 1) * 128],
                            x8_sb[k][:, 2 * j:2 * j + 2, :],
                            start=(j == 0), stop=(j == 2), perf_mode=PM)
                    ys.append(yt)
                # bf16 mlp chunks
                y3 = ypool.tile([128, w], F32, name="y3")
                y4 = ypool.tile([64, w], F32, name="y4")
                for i in range(6):
                    nc.tensor.matmul(y3[:, :], w1b[i][:, 0:128],
                                     xt_sb[k][i][:, :],
                                     start=(i == 0), stop=(i == 5))
                for i in range(6):
                    nc.tensor.matmul(y4[:, :], w1b[i][:, 128:192],
                                     xt_sb[k][i][:, :],
                                     start=(i == 0), stop=(i == 5))
                return ys + [y3, y4]

            def emit_middle(k, ys):
                nl = TILES[k][1]
                w = nl * NL
                loff = 1 if k == 0 else 0
                y0, y1, y2, y3, y4 = ys

                zf01 = zpool.tile([128, 2, 480], F8, name="zf01")
                zf2 = zpool.tile([128, 480], F8, name="zf2")
                zg0 = zpool.tile([128, 480], BF16, name="zg0")
                zg1 = zg1_t0 if k == 0 else zg1_rot[(k - 1) % 4]
                z_tiles[k] = (zf01, zf2, zg0, zg1)

                def v4(ap, p0, p1, cols):
                    return ap[p0:p1, 0:cols].rearrange(
                        "p (b l t) -> p b l t", b=2, t=T)

                # ---- conv_t chunk A (fc1 ca 0:128): fp8 accumulate ----
                za = zf01[:, 0, 0:w].rearrange("p (b l t) -> p b l t",
                                               b=2, t=T)
                yv0 = v4(y0, 0, 128, w)
                nc.scalar.activation(zf01[:, 0, 0:w], y0[:, :], AF.Identity,
                                     bias=col(3), scale=col(1))
                for b in (0, 1):
                    nc.vector.scalar_tensor_tensor(
                        out=za[:, b, :, 1:], in0=yv0[:, b, :, :T - 1],
                        scalar=col(0), in1=za[:, b, :, 1:],
                        op0=OP.mult, op1=OP.add)
                for b in (0, 1):
                    nc.vector.scalar_tensor_tensor(
                        out=za[:, b, :, :T - 1], in0=yv0[:, b, :, 1:],
                        scalar=col(2), in1=za[:, b, :, :T - 1],
                        op0=OP.mult, op1=OP.add)

                # ---- evict y1, pack both halves (b -> partitions) ----
                tmp1 = tpool.tile([128, 480], BF16, name="tmp1")
                nc.scalar.activation(tmp1[:, :w], y1[:, :], AF.Copy)
                pb1 = tpool.tile([128, 240], BF16, name="pb1")
                pb2 = tpool.tile([128, 240], BF16, name="pb2")
                hw = nl * T
                nc.gpsimd.dma_start(out=pb1[0:64, 0:hw], in_=tmp1[0:64, 0:hw])
                nc.gpsimd.dma_start(out=pb1[64:128, 0:hw],
                                    in_=tmp1[0:64, hw:2 * hw])
                nc.gpsimd.dma_start(out=pb2[0:64, 0:hw],
                                    in_=tmp1[64:128, 0:hw])
                nc.gpsimd.dma_start(out=pb2[64:128, 0:hw],
                                    in_=tmp1[64:128, hw:2 * hw])

                # ---- conv_t chunk B on packed (fc1 ca 128:192) ----
                qc8 = tpool.tile([128, 240], F8, name="qc8")
                pb1v = pb1[:, 0:hw].rearrange("p (l t) -> p l t", t=T)
                qcv = qc8[:, 0:hw].rearrange("p (l t) -> p l t", t=T)
                nc.scalar.activation(qc8[:, 0:hw], pb1[:, 0:hw], AF.Identity,
                                     bias=col(7), scale=col(5))
                nc.vector.scalar_tensor_tensor(
                    out=qcv[:, :, 1:], in0=pb1v[:, :, :T - 1],
                    scalar=col(4), in1=qcv[:, :, 1:], op0=OP.mult, op1=OP.add)
                nc.vector.scalar_tensor_tensor(
                    out=qcv[:, :, :T - 1], in0=pb1v[:, :, 1:],
                    scalar=col(6), in1=qcv[:, :, :T - 1],
                    op0=OP.mult, op1=OP.add)
                # unpack conv_t-B into zf01 ktile1 rows 0:64
                zk1 = zf01[:, 1, 0:w]
                zk1v = zk1.rearrange("p (b l t) -> p b l t", b=2, t=T)
                nc.gpsimd.dma_start(out=zk1[0:64, 0:hw], in_=qc8[0:64, 0:hw])
                nc.gpsimd.dma_start(out=zk1[0:64, hw:2 * hw],
                                    in_=qc8[64:128, 0:hw])

                # ---- gelu ----
                nc.scalar.activation(zg0[:, 0:w], y3[:, :], AF.Silu,
                                     bias=col(8), scale=1.702)
                nc.scalar.activation(zg1[0:64, 0:w], y4[:, :], AF.Silu,
                                     bias=col(9, 0, 64), scale=1.702)

                # ---- temporal diff -> padded grids (h-row k) ----
                lp0 = (k + 1) * PADW + 1
                tmp2 = tpool.tile([128, 480], BF16, name="tmp2")
                nc.scalar.activation(tmp2[:, :w], y2[:, :], AF.Copy)
                t2v = v4(tmp2, 0, 128, w)[:, :, loff:, :]
                pvU = puv[:, :, lp0:lp0 + HGRID, :]
                pb2v = pb2[:, 0:hw].rearrange("p (l t) -> p l t", t=T)
                pvP = ppv[:, lp0:lp0 + HGRID, :]
                for b in (0, 1):
                    nc.vector.scalar_tensor_tensor(
                        out=pvU[:, b, :, 1:], in0=t2v[:, b, :, 1:],
                        scalar=col(10), in1=t2v[:, b, :, :T - 1],
                        op0=OP.add, op1=OP.subtract)
                nc.vector.scalar_tensor_tensor(
                    out=pvP[:, :, 1:], in0=pb2v[:, loff:, 1:],
                    scalar=col(11), in1=pb2v[:, loff:, :T - 1],
                    op0=OP.add, op1=OP.subtract)

                # tile-0: zero the CLS cols of the offset-branch z rows
                if k == 0:
                    nc.vector.memset(zk1v[64:128, :, 0:1, :], 0.0)
                    zf2v = zf2[:, 0:w].rearrange("p (b l t) -> p b l t",
                                                 b=2, t=T)
                    nc.vector.memset(zf2v[:, :, 0:1, :], 0.0)

            def emit_spconv_B_out(j):
                nl = TILES[j][1]
                w = nl * NL
                hw = nl * T
                c0 = TILE_C0[j]
                loff = 1 if j == 0 else 0
                lp0 = (j + 1) * PADW + 1
                zf01, zf2, zg0, zg1 = z_tiles[j]

                # ---- spconv chunk U (off ca 64:192) -> zf2 ----
                ztU = tpool.tile([128, 448], BF16, name="ztU")
                zUv = ztU[:, 0:2 * HGRID * T].rearrange(
                    "p (b l t) -> p b l t", b=2, t=T)
                zf2v = zf2[:, 0:w].rearrange("p (b l t) -> p b l t",
                                             b=2, t=T)[:, :, loff:, :]
                for i, (dh, dw) in enumerate(TAPS):
                    sl = lp0 + dh * PADW + dw
                    pv = puv[:, :, sl:sl + HGRID, :]
                    if i == 0:
                        nc.vector.tensor_scalar(
                            out=zUv, in0=pv, scalar1=col(12), scalar2=None,
                            op0=OP.mult)
                    elif i < 8:
                        nc.vector.scalar_tensor_tensor(
                            out=zUv, in0=pv, scalar=col(12 + i), in1=zUv,
                            op0=OP.mult, op1=OP.add)
                    else:
                        nc.vector.scalar_tensor_tensor(
                            out=zf2v, in0=pv, scalar=col(12 + i), in1=zUv,
                            op0=OP.mult, op1=OP.add)
                # ---- spconv chunk P (off ca 0:64, packed) -> qs8 ----
                ztP = tpool.tile([128, 224], BF16, name="ztP")
                qs8 = tpool.tile([128, 224], F8, name="qs8")
                zPv = ztP[:, :].rearrange("p (l t) -> p l t", t=T)
                qsv = qs8[:, :].rearrange("p (l t) -> p l t", t=T)
                for i, (dh, dw) in enumerate(TAPS):
                    sl = lp0 + dh * PADW + dw
                    pv = ppv[:, sl:sl + HGRID, :]
                    if i == 0:
                        nc.vector.tensor_scalar(
                            out=zPv, in0=pv, scalar1=col(21), scalar2=None,
                            op0=OP.mult)
                    elif i < 8:
                        nc.vector.scalar_tensor_tensor(
                            out=zPv, in0=pv, scalar=col(21 + i), in1=zPv,
                            op0=OP.mult, op1=OP.add)
                    else:
                        nc.vector.scalar_tensor_tensor(
                            out=qsv, in0=pv, scalar=col(21 + i), in1=zPv,
                            op0=OP.mult, op1=OP.add)
                # unpack spconv-P into zf01 ktile1 rows 64:128 (l>=loff cols)
                zk1 = zf01[:, 1, 0:w].rearrange("p (b l t) -> p b l t",
                                                b=2, t=T)
                q4 = qs8[:, :].rearrange("p (o l t) -> p o l t", o=1, t=T)
                nc.gpsimd.dma_start(out=zk1[64:128, 0:1, loff:, :],
                                    in_=q4[0:64])
                nc.gpsimd.dma_start(out=zk1[64:128, 1:2, loff:, :],
                                    in_=q4[64:128])

                # ---- matmul B + eviction + store ----
                for m in range(6):
                    m0 = m * 128
                    ot = opool.tile([128, w], F32, name="ops")
                    nc.tensor.matmul(ot[:, :], w2a[:, :, m0:m0 + 128],
                                     zf01[:, :, 0:w], start=True, stop=False,
                                     perf_mode=PM)
                    nc.tensor.matmul(ot[:, :], w2b8[:, m0:m0 + 128],
                                     zf2[:, 0:w], start=False, stop=False)
                    nc.tensor.matmul(ot[:, :], w2g0[:, m0:m0 + 128],
                                     zg0[:, 0:w], start=False, stop=False)
                    nc.tensor.matmul(ot[:, :], w2g1[:, m0:m0 + 128],
                                     zg1[0:67, 0:w], start=False, stop=True)
                    osb = opool_sb.tile([128, w], BF16, name="osb")
                    nc.scalar.activation(osb[:, :], ot[:, :], AF.Identity,
                                         bias=col(32 + m), scale=1.0 / G)
                    nc.sync.dma_start(out=out_d[m0:m0 + 128, c0:c0 + w],
                                      in_=osb[:, :])

            for k in range(14):
                ys = emit_A(k)
                if k >= 3:
                    emit_spconv_B_out(k - 3)
                if k == 13:
                    emit_spconv_B_out(11)
                emit_middle(k, ys)
            emit_spconv_B_out(12)
            emit_spconv_B_out(13)

    nc.compile()
    return nc


_cached = {}


def _get_kernel(diff_bias_zero=True):
    key = ("nc", diff_bias_zero)
    if key not in _cached:
        _cached[key] = build_kernel(diff_bias_zero)
    return _cached[key]


def _host_xt(x):
    """x (L, 256, C) f32 -> (8, C, NCOLS) f32, per-tile b-major token order."""
    out = np.empty((NCORES, C, NCOLS), np.float32)
    x5 = x.reshape(L, NCORES, 2, T, C)
    for k, (l0, nl) in enumerate(TILES):
        blk = x5[l0:l0 + nl]                      # (nl, 8, 2, T, C)
        blk = blk.transpose(1, 4, 2, 0, 3)        # (8, C, 2, nl, T)
        out[:, :, TILE_C0[k]:TILE_C0[k + 1]] = blk.reshape(NCORES, C, nl * NL)
    return out


def _host_out(outT):
    """outT (8, C, NCOLS) -> out (L, 256, C) f32."""
    out = np.empty((L, NCORES * NL, C), np.float32)
    for k, (l0, nl) in enumerate(TILES):
        blk = outT[:, :, TILE_C0[k]:TILE_C0[k + 1]].astype(np.float32)
        blk = blk.reshape(NCORES, C, 2, nl, T)    # (8, C, 2, nl, T)
        blk = blk.transpose(3, 0, 2, 4, 1)        # (nl, 8, 2, T, C)
        out[l0:l0 + nl] = blk.reshape(nl, NCORES * NL, C)
    return out


def kernel(x, T, fc1_w, fc1_b, conv_w, conv_b, fc2_w, fc2_b,
           off_fc1_w, off_fc1_b, off_conv_w, off_conv_b, off_fc2_w, off_fc2_b,
           mlp_in_w, mlp_in_b, mlp_out_w, mlp_out_b):
    bf = ml_dtypes.bfloat16
    f8 = ml_dtypes.float8_e4m3fn
    x = np.asarray(x, np.float32)
    to_np = lambda a: np.asarray(a, np.float32)
    (fc1_w, fc1_b, conv_w, conv_b, fc2_w, fc2_b, off_fc1_w, off_fc1_b,
     off_conv_w, off_conv_b, off_fc2_w, off_fc2_b, mlp_in_w, mlp_in_b,
     mlp_out_w, mlp_out_b) = map(to_np, (
        fc1_w, fc1_b, conv_w, conv_b, fc2_w, fc2_b, off_fc1_w, off_fc1_b,
        off_conv_w, off_conv_b, off_fc2_w, off_fc2_b, mlp_in_w, mlp_in_b,
        mlp_out_w, mlp_out_b))

    xtf = _host_xt(x)                       # (8, C, NCOLS) f32
    xt = xtf.astype(bf)
    x8 = np.ascontiguousarray(
        xtf.astype(f8).reshape(NCORES, 6, 128, NCOLS).transpose(0, 2, 1, 3))

    w1cat = np.concatenate([fc1_w, off_fc1_w], axis=1)      # (768, 384)
    w18 = np.ascontiguousarray(
        (SW * w1cat).astype(f8).reshape(6, 128, 384).transpose(1, 0, 2))
    w1b = mlp_in_w.astype(bf)                               # (768, 192)

    w2kt0 = SW * fc2_w[0:128]
    w2kt1 = np.concatenate([SW * fc2_w[128:192], SW * off_fc2_w[0:64]], 0)
    w2a = np.ascontiguousarray(
        np.stack([w2kt0, w2kt1], 0).astype(f8).transpose(1, 0, 2))
    w2b8 = (SW * off_fc2_w[64:192]).astype(f8)

    w2g0 = (G / 1.702 * mlp_out_w[0:128]).astype(bf)
    w0, w1c, w2c = conv_w[:, 0, 0], conv_w[:, 0, 1], conv_w[:, 0, 2]
    aux_w2 = np.stack([
        G * (off_conv_b @ off_fc2_w + off_fc2_b),
        G * ((-w0 * fc1_b) @ fc2_w),
        G * ((-w2c * fc1_b) @ fc2_w),
    ], 0)
    w2g1 = np.concatenate([G / 1.702 * mlp_out_w[128:192], aux_w2],
                          0).astype(bf)

    scal = _pack_scalars(conv_w, conv_b, fc1_b, mlp_in_b, off_fc1_b,
                         off_conv_w, fc2_b, mlp_out_b)
    auxz = _aux_patterns()

    nc = _get_kernel(diff_bias_zero=not np.any(off_fc1_b))
    in_maps = [{"xt": xt[i], "x8": x8[i], "w1b": w1b, "w18": w18,
                "w2a": w2a, "w2b8": w2b8, "w2g0": w2g0, "w2g1": w2g1,
                "auxz": auxz, "scal": scal}
               for i in range(NCORES)]
    res = bass_utils.run_bass_kernel_spmd(nc, in_maps,
                                          core_ids=list(range(NCORES)))
    _cached["last_result"] = res

    outT = np.stack([np.asarray(res.results[i]["out"]) for i in range(NCORES)])
    return np.ascontiguousarray(_host_out(outT))

